# revision 32
# baseline (speedup 1.0000x reference)
"""MinimalRNNCell on 8 Trainium2 NeuronCores.

h_t = x_t @ W + h_{t-1} @ R, h_0 = 0, for x: [B=32, T=1024, D=512],
W: [D, U=512], R: [U, U]. Returns all h_t -> [B, T, U] float32.

Strategy (data-parallel over batch; chunked linear scan in a
block-diagonalizing eigenbasis):
  - Host: eigendecompose R (fp64), group the spectrum into 4 conjugate-
    closed clusters of 128 dims (sorted by eigenvalue angle), take an
    orthonormal basis Q_g of each invariant subspace. P = [Q_1..Q_4]
    gives D = P^-1 R P exactly block-diagonal with dense 128x128 blocks
    and cond(P) ~ 2e2 (vs ~5e3 for the full modal form, which fails
    tf32 accuracy). The scan runs in z = h @ P coordinates:
        z_t = x_t @ (W P) + z_{t-1} @ D,   h_t = z_t @ P^-1.
    Recurrence matmuls touch only the 4 diagonal blocks -> 4x cheaper
    than dense R, at the cost of one extra dense transform (z @ P^-1)
    fused into the output drain. Net PE work: 324 vs 400 [128x128x512]
    matmuls for the dense-R chunked scan.
  - Shard batch over 8 cores (4 rows each). All matmul work runs in the
    transposed layout z^T[U, r]; host pre-permutes x into xr[c, d, r]
    with r = (chunk j, batch b), t = j*C + c, so every DMA is
    contiguous.
  - Phase A: C=8 sequential steps; step c advances all L=128 chunks at
    once: zloc_c = x_c @ (WP) + zloc_{c-1} @ D as one PSUM group per
    128-row block (4 WP matmuls + 1 diag-block matmul).
  - Phase B: chunk-boundary carries in ONE round of npow taps,
    carry_j = end_j + sum_k end_{j-k} @ D^(C*k), with host powers
    (block-diagonal -> 1 matmul per tap per 128-block, no serial
    doubling rounds). Contractive spectrum -> ~3 taps survive POW_TOL.
  - Phase C: C steps; correction carry @ D^(c+1) via host powers (all
    steps independent - no serial G chain), z_c = zloc_c + corr fused
    on DVE, then the dense h_c = z_c @ P^-1 transform streams out.

Scheduling notes (from NTFF device traces): every input tensor is ONE
partition-major flat DMA (a stalled dma_start on a compute queue
head-of-line blocks the copies phase B needs -> 9us PE bubble + HAM
re-throttle); PSUM->SBUF copies that gate matmuls live on the vector
queue only; the transform drain is software-pipelined one step behind
the correction matmuls. fp16 throughout (same 10-bit mantissa as tf32
here, half the DMA bytes, FWL halves LDWEIGHTS, 2x DVE).

Matmul dtype is selectable via RNN_MM_DTYPE: "f32" (exact, 4 cyc/row),
"f32r" (TF32), "f16" (default), "bf16".
"""

import os

import numpy as np

import concourse.bass as bass
import concourse.mybir as mybir
import concourse.tile as tile
from concourse import bass_utils

B, T, D, U = 32, 1024, 512, 512
NCORES = 8
BLOC = B // NCORES  # 4 batch rows per core
C = 8  # intra-chunk steps (phase A/C length)
L = T // C  # 128 chunks
RCOLS = BLOC * L  # 512 moving columns
NCH = U // 128  # 4 partition chunks of the 512-dim
POW_TOL = 1e-4  # drop Kogge-Stone rounds with ||D^(C*2^k)||_2 below this
MAX_SYNC_WAITS = 1

MM_DTYPE = os.environ.get("RNN_MM_DTYPE", "f16")
# debug: which phases to build ("a", "ab", "abc" = full kernel)
PHASES = os.environ.get("RNN_PHASES", "abc")


def _split_sync_waits(nc, max_waits=MAX_SYNC_WAITS):
    """Walrus rejects instructions carrying more than a couple of sync
    waits (CTRL structs in this toolchain cap out below what Tile's
    final drain needs). Hoist excess waits onto single-wait NoOps
    placed immediately before the offending instruction."""
    for fn in nc.m.functions:
        for bb in fn.blocks:
            insts = bb.instructions
            out, changed = [], False
            for inst in insts:
                si = inst.sync_info
                waits = list(si.on_wait) if si is not None else []
                if len(waits) > max_waits:
                    for k, w in enumerate(waits[:-max_waits]):
                        out.append(
                            mybir.InstNoOp(
                                name=f"I-wsplit-{inst.name}-{k}",
                                engine=inst.engine,
                                ins=[],
                                outs=[],
                                sync_info=mybir.SyncInfo(on_wait=[w], on_update=[]),
                            )
                        )
                    inst.sync_info = mybir.SyncInfo(
                        on_wait=waits[-max_waits:], on_update=list(si.on_update)
                    )
                    changed = True
                out.append(inst)
            if changed:
                insts[:] = out


def _build_nc(npow, reps=1):
    f32 = mybir.dt.float32
    if MM_DTYPE == "bf16":
        io_dt = mybir.dt.bfloat16
    elif MM_DTYPE == "f16":
        io_dt = mybir.dt.float16
    elif MM_DTYPE == "f32r":
        io_dt = mybir.dt.float32r
    else:
        io_dt = f32

    def vin(ap):
        # DVE/ACT read of an f32r tile: same bits as f32
        return ap.bitcast(f32) if MM_DTYPE == "f32r" else ap

    nc = bass.Bass("TRN2", target_bir_lowering=False, debug=False)
    # all inputs partition-major and pre-flattened so each tensor (and each
    # x step) is ONE wide DMA: few triggers, no queue head-of-line blocking
    xr_d = nc.dram_tensor("xr", [C, 128, NCH * RCOLS], io_dt, kind="ExternalInput").ap()
    wp_d = nc.dram_tensor("wp", [128, NCH * U], io_dt, kind="ExternalInput").ap()
    pv_d = nc.dram_tensor("pv", [128, NCH * U], io_dt, kind="ExternalInput").ap()
    db_d = nc.dram_tensor("db", [128, NCH * 128], io_dt, kind="ExternalInput").ap()
    dcp_d = nc.dram_tensor(
        "dcp", [128, C * NCH * 128], io_dt, kind="ExternalInput"
    ).ap()
    if npow:
        pw_d = nc.dram_tensor(
            "pows", [128, npow * NCH * 128], io_dt, kind="ExternalInput"
        ).ap()
    hr_d = nc.dram_tensor("hr", [C, 128, NCH * RCOLS], f32, kind="ExternalOutput").ap()

    # zero-pad in front of the chunk axis so the multi-tap shifted reads in
    # phase B (shift up to npow chunks) and the carry shift in phase C fall
    # into zeros instead of needing edge cases
    pad = BLOC * max(npow, 1)

    # pools shared across reps: identical per-rep instruction stream, but
    # the scheduler can overlap rep r+1's prefetch DMAs with rep r's tail
    with tile.TileContext(nc) as tc:
      with (
            tc.tile_pool(name="wts", bufs=64) as wpool,
            tc.tile_pool(name="hl", bufs=C * NCH) as hlpool,
            tc.tile_pool(name="xt", bufs=2 * NCH) as xtpool,
            tc.tile_pool(name="hp", bufs=2 * NCH) as hppool,
            tc.tile_pool(name="z", bufs=2 * NCH) as zpool,
            tc.tile_pool(name="out", bufs=2 * NCH) as outpool,
            tc.tile_pool(name="ps", bufs=8, space="PSUM") as pspool,
        ):
       for _rep in range(reps):
            # --- resident weights: ONE wide DMA per tensor; views slice out
            # the stationary 128x128 blocks ---
            def load_flat(src, name, ncols):
                t = wpool.tile([128, ncols], io_dt, tag=name, bufs=1)
                nc.sync.dma_start(out=t[:], in_=src[:, :])
                return t

            def band_views(t):
                # t[:, a*U + b*128 : ...] = block (row-band a, col-block b)
                return [
                    [t[:, a * U + 128 * b : a * U + 128 * (b + 1)] for b in range(NCH)]
                    for a in range(NCH)
                ]

            def load_x(c):
                t = xtpool.tile([128, NCH * RCOLS], io_dt, tag="x", bufs=3)
                nc.sync.dma_start(out=t[:], in_=xr_d[c])
                return [t[:, RCOLS * d : RCOLS * (d + 1)] for d in range(NCH)]

            wp_t = band_views(load_flat(wp_d, "wp", NCH * U))
            dbt = load_flat(db_d, "db", NCH * 128)
            d_t = [dbt[:, 128 * u : 128 * (u + 1)] for u in range(NCH)]
            xpre = {0: load_x(0), 1: load_x(1)}
            pv_t = band_views(load_flat(pv_d, "pv", NCH * U))
            pw_t = []
            dcp_t = []

            # --- phase A: intra-chunk local scan in z-coordinates ---
            hl = [[None] * NCH for _ in range(C)]
            for c in range(C):
                xts = xpre.pop(c) if c in xpre else load_x(c)
                if c == 2 and npow:
                    pwt = load_flat(pw_d, "pw", npow * NCH * 128)
                    pw_t = [
                        [
                            pwt[:, (k * NCH + u) * 128 : (k * NCH + u + 1) * 128]
                            for u in range(NCH)
                        ]
                        for k in range(npow)
                    ]
                if c == 3:
                    # phase-C correction powers D^(c+1), diagonal blocks
                    dcpt = load_flat(dcp_d, "dcp", C * NCH * 128)
                    dcp_t = [
                        [
                            dcpt[:, (cc * NCH + u) * 128 : (cc * NCH + u + 1) * 128]
                            for u in range(NCH)
                        ]
                        for cc in range(C)
                    ]
                for u in range(NCH):
                    ops = [(wp_t[d][u][:], xts[d][:]) for d in range(NCH)]
                    if c > 0:
                        ops.append((d_t[u][:], hl[c - 1][u][:]))
                    ps = pspool.tile([128, RCOLS], f32, tag="ps")
                    for i, (lhsT, rhs) in enumerate(ops):
                        nc.tensor.matmul(
                            ps[:], lhsT, rhs,
                            start=(i == 0), stop=(i == len(ops) - 1),
                        )
                    # critical-path copies stay on DVE only: the scalar queue
                    # stalls on DMA-ring backpressure from its dma_starts
                    ht = hlpool.tile([128, RCOLS], io_dt, tag=f"hl{c}_{u}", bufs=1)
                    nc.vector.tensor_copy(out=ht[:], in_=ps[:])
                    hl[c][u] = ht

            if PHASES == "a":
                # debug build: dump zloc as the output, skip B/C
                for c in range(C):
                    ot = outpool.tile([128, NCH * RCOLS], f32, tag="o", bufs=2)
                    for u in range(NCH):
                        nc.vector.tensor_copy(
                            out=ot[:, RCOLS * u : RCOLS * (u + 1)], in_=vin(hl[c][u][:])
                        )
                    nc.sync.dma_start(out=hr_d[c], in_=ot[:])
            else:
                # --- phase B: chunk-end carries, single round of npow taps:
                # carry_j = end_j + sum_k end_{j-k} @ D^(C*k)  (k=1..npow;
                # dropped tail has ||D^(C*(npow+1))|| <= POW_TOL) ---
                hpa, hpb = [], []
                for v in range(NCH):
                    ta = hppool.tile([128, pad + RCOLS], io_dt, tag=f"hpa{v}", bufs=1)
                    tb = hppool.tile([128, BLOC + RCOLS], io_dt, tag=f"hpb{v}", bufs=1)
                    nc.gpsimd.memset(vin(ta[:, 0:pad]), 0.0)
                    nc.gpsimd.memset(vin(tb[:, 0:BLOC]), 0.0)
                    nc.vector.tensor_copy(
                        out=ta[:, pad : pad + RCOLS], in_=vin(hl[C - 1][v][:])
                    )
                    hpa.append(ta)
                    hpb.append(tb)
                for u in range(NCH):
                    ps = pspool.tile([128, RCOLS], f32, tag="ps")
                    for k in range(npow):
                        sh = BLOC * (k + 1)
                        nc.tensor.matmul(
                            ps[:], pw_t[k][u][:],
                            hpa[u][:, pad - sh : pad - sh + RCOLS],
                            start=(k == 0), stop=(k == npow - 1),
                        )
                    nc.vector.tensor_add(
                        out=hpb[u][:, BLOC : BLOC + RCOLS], in0=ps[:],
                        in1=vin(hpa[u][:, pad : pad + RCOLS]),
                    )
                src = hpb

                if PHASES == "ab":
                    for c in range(C):
                        ot = outpool.tile([128, NCH * RCOLS], f32, tag="o", bufs=2)
                        for u in range(NCH):
                            nc.vector.tensor_copy(
                                out=ot[:, RCOLS * u : RCOLS * (u + 1)],
                                in_=vin(hl[c][u][:]),
                            )
                        nc.sync.dma_start(out=hr_d[c], in_=ot[:])
                else:
                    # --- phase C: apply carries, transform, emit h ---
                    # carry views stay static; correction for step c is
                    # carry @ D^(c+1) via host powers (no serial G chain)
                    carry = [src[v][:, 0:RCOLS] for v in range(NCH)]
                    def drain(dc, pss):
                        # PSUM -> one wide SBUF tile -> single DRAM DMA
                        ot = outpool.tile([128, NCH * RCOLS], f32, tag="o", bufs=2)
                        for u in range(NCH):
                            dst = ot[:, RCOLS * u : RCOLS * (u + 1)]
                            if u < 2:
                                nc.vector.tensor_copy(out=dst, in_=pss[u][:])
                            else:
                                nc.scalar.copy(out=dst, in_=pss[u][:])
                        nc.sync.dma_start(out=hr_d[dc], in_=ot[:])

                    pend = None
                    for c in range(C):
                        zts = []
                        for u in range(NCH):
                            psg = pspool.tile([128, RCOLS], f32, tag="ps")
                            nc.tensor.matmul(
                                psg[:], dcp_t[c][u][:], carry[u],
                                start=True, stop=True,
                            )
                            # z_c = zloc_c + carry @ D^(c+1)
                            zt = zpool.tile([128, RCOLS], io_dt, tag=f"z{u}", bufs=2)
                            nc.vector.tensor_add(
                                out=zt[:], in0=psg[:], in1=vin(hl[c][u][:])
                            )
                            zts.append(zt)
                        # drain the PREVIOUS step's transform after this
                        # step's z-adds so DVE frees its PSUM banks in time
                        # without delaying the adds that gate the PE
                        if pend is not None:
                            drain(*pend)
                        pss = []
                        for u in range(NCH):
                            ps = pspool.tile([128, RCOLS], f32, tag="ps")
                            for v in range(NCH):
                                nc.tensor.matmul(
                                    ps[:], pv_t[v][u][:], zts[v][:],
                                    start=(v == 0), stop=(v == NCH - 1),
                                )
                            pss.append(ps)
                        pend = (c, pss)
                    drain(*pend)

    _split_sync_waits(nc)
    return nc


_CACHE = {}


def _get_nc(npow, reps=1):
    key = (npow, MM_DTYPE, PHASES, reps)
    if key not in _CACHE:
        _CACHE[key] = _build_nc(npow, reps)
    return _CACHE[key]


def _tf32_round(a):
    b = np.ascontiguousarray(a, np.float32).view(np.uint32)
    r = ((b >> np.uint32(13)) & np.uint32(1)) + np.uint32(0x0FFF)
    b = (b + r) & np.uint32(0xFFFFE000)
    return b.view(np.float32)


def _cast_host(a):
    if MM_DTYPE == "bf16":
        import ml_dtypes

        return np.ascontiguousarray(a.astype(ml_dtypes.bfloat16))
    if MM_DTYPE == "f16":
        return np.ascontiguousarray(np.asarray(a, np.float32).astype(np.float16))
    if MM_DTYPE == "f32r":
        return np.ascontiguousarray(_tf32_round(a))
    return np.ascontiguousarray(a.astype(np.float32))


def _block_diagonalize(r):
    """Split spec(R) into 4 conjugate-closed clusters of 128 dims (by
    eigenvalue angle) and orthonormalize each invariant subspace.
    Returns (P, Pinv, Dbd) fp64 with Dbd = P^-1 R P exactly
    block-diagonal (dense 128x128 diagonal blocks)."""
    r = np.asarray(r, np.float64)
    lam, V = np.linalg.eig(r)
    modes = []
    for i in range(len(lam)):
        if abs(lam[i].imag) < 1e-12:
            modes.append((1, 0.0 if lam[i].real > 0 else np.pi, [V[:, i].real]))
        elif lam[i].imag > 0:
            modes.append(
                (2, abs(np.angle(lam[i])), [V[:, i].real, V[:, i].imag])
            )
    modes.sort(key=lambda m: m[1])
    remaining = list(modes)
    clusters = []
    for g in range(NCH):
        cl, dsum = [], 0
        i = 0
        while dsum < 128 and i <= len(remaining):
            if i == len(remaining):
                break
            m = remaining[i]
            if dsum + m[0] <= 128:
                cl.append(m)
                dsum += m[0]
                remaining.pop(i)
            else:
                j = next(
                    (jj for jj in range(i, len(remaining)) if remaining[jj][0] == 1),
                    None,
                )
                if j is None:
                    break
                cl.append(remaining[j])
                dsum += 1
                remaining.pop(j)
        assert dsum == 128, (g, dsum)
        clusters.append(cl)
    assert not remaining
    Qs = []
    for cl in clusters:
        cols = np.stack([col for m in cl for col in m[2]], axis=1)
        q, _ = np.linalg.qr(cols)
        Qs.append(q)
    P = np.concatenate(Qs, axis=1)
    Pinv = np.linalg.inv(P)
    Dfull = Pinv @ r @ P
    Dbd = np.zeros_like(Dfull)
    for a in range(NCH):
        s = slice(128 * a, 128 * (a + 1))
        Dbd[s, s] = Dfull[s, s]
    return P, Pinv, Dbd


def prepare_inputs(x, kernel, recurrent_kernel):
    """Host-side decomposition + shard + permute. Returns (in_maps, npow)."""
    x = np.asarray(x)
    kernel = np.asarray(kernel, np.float64)
    P, Pinv, Dbd = _block_diagonalize(recurrent_kernel)
    def flat_bands(mat):
        # [D, U] -> [128, (D/128)*U]: row-bands side by side, partition-major
        return np.concatenate(
            [mat[128 * a : 128 * (a + 1), :] for a in range(mat.shape[0] // 128)],
            axis=1,
        )

    def flat_diag(mat):
        # diagonal 128-blocks side by side -> [128, NCH*128]
        return np.concatenate(
            [
                mat[128 * u : 128 * (u + 1), 128 * u : 128 * (u + 1)]
                for u in range(NCH)
            ],
            axis=1,
        )

    # multi-tap carry powers D^(C*k), k=1..npow (block-diagonal); stop when
    # the next tap's norm is below tolerance (contractive spectrum).
    pows = []
    dC = np.linalg.matrix_power(Dbd, C)
    m = dC
    while np.linalg.norm(m, 2) > POW_TOL and len(pows) < L - 1:
        pows.append(m)
        m = m @ dC
    npow = len(pows)
    pw = (
        _cast_host(np.concatenate([flat_diag(p) for p in pows], axis=1))
        if npow
        else None
    )
    wp = _cast_host(flat_bands(kernel @ P))
    pv = _cast_host(flat_bands(Pinv))
    db = _cast_host(flat_diag(Dbd))
    # phase-C correction powers D^(c+1), c = 0..C-1 (block-diagonal)
    dcp = _cast_host(
        np.concatenate(
            [flat_diag(np.linalg.matrix_power(Dbd, c + 1)) for c in range(C)], axis=1
        )
    )
    in_maps = []
    for k in range(NCORES):
        xc = x[BLOC * k : BLOC * (k + 1)]  # [BLOC, T, D]
        # xr[c, p, d*RCOLS + j*BLOC + b] = xc[b, j*C + c, d*128 + p]
        xr = _cast_host(
            xc.reshape(BLOC, L, C, NCH, 128)
            .transpose(2, 4, 3, 1, 0)
            .reshape(C, 128, NCH * RCOLS)
        )
        im = {"xr": xr, "wp": wp, "pv": pv, "db": db, "dcp": dcp}
        if npow:
            im["pows"] = pw
        in_maps.append(im)
    return in_maps, npow


def assemble_output(results):
    out = np.empty((B, T, U), np.float32)
    for k in range(NCORES):
        hr = results[k]["hr"]  # [C, 128, NCH*RCOLS]
        # out[b, j*C + c, u*128 + p] = hr[c, p, u*RCOLS + j*BLOC + b]
        out[BLOC * k : BLOC * (k + 1)] = (
            hr.reshape(C, 128, NCH, L, BLOC)
            .transpose(4, 3, 0, 2, 1)
            .reshape(BLOC, T, U)
        )
    return out


_RUNNERS = {}


def _get_runner(nc):
    """Build (once) a sharded jitted executable for `nc` on 8 cores.
    Mirrors bass2jax.run_bass_via_pjrt's multi-core path, but cached so
    repeated kernel() calls don't re-trace/re-compile."""
    if nc in _RUNNERS:
        return _RUNNERS[nc]
    import jax
    from jax.sharding import Mesh, PartitionSpec
    from jax.experimental.shard_map import shard_map
    from concourse import bass2jax

    bass2jax.install_neuronx_cc_hook()
    partition_name = nc.partition_id_tensor.name if nc.partition_id_tensor else None
    in_names, out_names, out_avals = [], [], []
    for alloc in nc.m.functions[0].allocations:
        if not isinstance(alloc, mybir.MemoryLocationSet):
            continue
        name = alloc.memorylocations[0].name
        if alloc.kind == "ExternalInput":
            if name != partition_name:
                in_names.append(name)
        elif alloc.kind == "ExternalOutput":
            out_names.append(name)
            out_avals.append(
                jax.core.ShapedArray(
                    tuple(alloc.tensor_shape), mybir.dt.np(alloc.dtype)
                )
            )
    n_params = len(in_names)
    in_names_all = list(in_names) + out_names
    if partition_name is not None:
        in_names_all.append(partition_name)

    def _body(*args):
        operands = list(args)
        if partition_name is not None:
            operands.append(bass2jax.partition_id_tensor())
        return tuple(
            bass2jax._bass_exec_p.bind(
                *operands,
                out_avals=tuple(out_avals),
                in_names=tuple(in_names_all),
                out_names=tuple(out_names),
                lowering_input_output_aliases=(),
                sim_require_finite=True,
                sim_require_nnan=True,
                nc=nc,
            )
        )

    devices = jax.devices()[:NCORES]
    mesh = Mesh(np.asarray(devices), ("core",))
    nouts = len(out_names)
    sharded = jax.jit(
        shard_map(
            _body,
            mesh=mesh,
            in_specs=(PartitionSpec("core"),) * (n_params + nouts),
            out_specs=(PartitionSpec("core"),) * nouts,
            check_rep=False,
        ),
        keep_unused=True,
    )

    def run(in_maps):
        concat_in = [
            np.concatenate([np.asarray(in_maps[c][nm]) for c in range(NCORES)], axis=0)
            for nm in in_names
        ]
        concat_zero = [
            np.zeros((NCORES * a.shape[0], *a.shape[1:]), a.dtype) for a in out_avals
        ]
        outs = sharded(*concat_in, *concat_zero)
        return [
            {
                nm: np.asarray(outs[i]).reshape(NCORES, *out_avals[i].shape)[c]
                for i, nm in enumerate(out_names)
            }
            for c in range(NCORES)
        ]

    run.sharded = sharded
    run.in_names = list(in_names)
    run.out_shapes = [(tuple(a.shape), a.dtype) for a in out_avals]
    _RUNNERS[nc] = run
    return run


def kernel(x, kernel, recurrent_kernel):
    in_maps, npow = prepare_inputs(x, kernel, recurrent_kernel)
    nc = _get_nc(npow)
    results = _get_runner(nc)(in_maps)
    return assemble_output(results)


# revision 34
# speedup vs baseline: 1.0039x; 1.0039x over previous
"""MinimalRNNCell on 8 Trainium2 NeuronCores.

h_t = x_t @ W + h_{t-1} @ R, h_0 = 0, for x: [B=32, T=1024, D=512],
W: [D, U=512], R: [U, U]. Returns all h_t -> [B, T, U] float32.

Strategy (data-parallel over batch; chunked linear scan in a
block-diagonalizing eigenbasis):
  - Host: eigendecompose R (fp64), group the spectrum into 4 conjugate-
    closed clusters of 128 dims (sorted by eigenvalue angle), take an
    orthonormal basis Q_g of each invariant subspace. P = [Q_1..Q_4]
    gives D = P^-1 R P exactly block-diagonal with dense 128x128 blocks
    and cond(P) ~ 2e2 (vs ~5e3 for the full modal form, which fails
    tf32 accuracy). The scan runs in z = h @ P coordinates:
        z_t = x_t @ (W P) + z_{t-1} @ D,   h_t = z_t @ P^-1.
    Recurrence matmuls touch only the 4 diagonal blocks -> 4x cheaper
    than dense R, at the cost of one extra dense transform (z @ P^-1)
    fused into the output drain. Net PE work: 324 vs 400 [128x128x512]
    matmuls for the dense-R chunked scan.
  - Shard batch over 8 cores (4 rows each). All matmul work runs in the
    transposed layout z^T[U, r]; host pre-permutes x into xr[c, d, r]
    with r = (chunk j, batch b), t = j*C + c, so every DMA is
    contiguous.
  - Phase A: C=8 sequential steps; step c advances all L=128 chunks at
    once: zloc_c = x_c @ (WP) + zloc_{c-1} @ D as one PSUM group per
    128-row block (4 WP matmuls + 1 diag-block matmul).
  - Phase B: chunk-boundary carries in ONE round of npow taps,
    carry_j = end_j + sum_k end_{j-k} @ D^(C*k), with host powers
    (block-diagonal -> 1 matmul per tap per 128-block, no serial
    doubling rounds). Contractive spectrum -> ~3 taps survive POW_TOL.
  - Phase C: C steps; correction carry @ D^(c+1) via host powers (all
    steps independent - no serial G chain), z_c = zloc_c + corr fused
    on DVE, then the dense h_c = z_c @ P^-1 transform streams out.

Scheduling notes (from NTFF device traces): every input tensor is ONE
partition-major flat DMA (a stalled dma_start on a compute queue
head-of-line blocks the copies phase B needs -> 9us PE bubble + HAM
re-throttle); PSUM->SBUF copies that gate matmuls live on the vector
queue only; the transform drain is software-pipelined one step behind
the correction matmuls. fp16 throughout (same 10-bit mantissa as tf32
here, half the DMA bytes, FWL halves LDWEIGHTS, 2x DVE).

Matmul dtype is selectable via RNN_MM_DTYPE: "f32" (exact, 4 cyc/row),
"f32r" (TF32), "f16" (default), "bf16".
"""

import os

import numpy as np

import concourse.bass as bass
import concourse.mybir as mybir
import concourse.tile as tile
from concourse import bass_utils

B, T, D, U = 32, 1024, 512, 512
NCORES = 8
BLOC = B // NCORES  # 4 batch rows per core
C = 8  # intra-chunk steps (phase A/C length)
L = T // C  # 128 chunks
RCOLS = BLOC * L  # 512 moving columns
NCH = U // 128  # 4 partition chunks of the 512-dim
POW_TOL = 3e-4  # drop carry taps with ||D^(C*k)||_2 below this
MAX_SYNC_WAITS = 1

MM_DTYPE = os.environ.get("RNN_MM_DTYPE", "f16")
# debug: which phases to build ("a", "ab", "abc" = full kernel)
PHASES = os.environ.get("RNN_PHASES", "abc")


def _split_sync_waits(nc, max_waits=MAX_SYNC_WAITS):
    """Walrus rejects instructions carrying more than a couple of sync
    waits (CTRL structs in this toolchain cap out below what Tile's
    final drain needs). Hoist excess waits onto single-wait NoOps
    placed immediately before the offending instruction."""
    for fn in nc.m.functions:
        for bb in fn.blocks:
            insts = bb.instructions
            out, changed = [], False
            for inst in insts:
                si = inst.sync_info
                waits = list(si.on_wait) if si is not None else []
                if len(waits) > max_waits:
                    for k, w in enumerate(waits[:-max_waits]):
                        out.append(
                            mybir.InstNoOp(
                                name=f"I-wsplit-{inst.name}-{k}",
                                engine=inst.engine,
                                ins=[],
                                outs=[],
                                sync_info=mybir.SyncInfo(on_wait=[w], on_update=[]),
                            )
                        )
                    inst.sync_info = mybir.SyncInfo(
                        on_wait=waits[-max_waits:], on_update=list(si.on_update)
                    )
                    changed = True
                out.append(inst)
            if changed:
                insts[:] = out


def _build_nc(npow, reps=1):
    f32 = mybir.dt.float32
    if MM_DTYPE == "bf16":
        io_dt = mybir.dt.bfloat16
    elif MM_DTYPE == "f16":
        io_dt = mybir.dt.float16
    elif MM_DTYPE == "f32r":
        io_dt = mybir.dt.float32r
    else:
        io_dt = f32

    def vin(ap):
        # DVE/ACT read of an f32r tile: same bits as f32
        return ap.bitcast(f32) if MM_DTYPE == "f32r" else ap

    nc = bass.Bass("TRN2", target_bir_lowering=False, debug=False)
    # all inputs partition-major and pre-flattened so each tensor (and each
    # x step) is ONE wide DMA: few triggers, no queue head-of-line blocking
    xr_d = nc.dram_tensor("xr", [C, 128, NCH * RCOLS], io_dt, kind="ExternalInput").ap()
    wp_d = nc.dram_tensor("wp", [128, NCH * U], io_dt, kind="ExternalInput").ap()
    pv_d = nc.dram_tensor("pv", [128, NCH * U], io_dt, kind="ExternalInput").ap()
    db_d = nc.dram_tensor("db", [128, NCH * 128], io_dt, kind="ExternalInput").ap()
    dcp_d = nc.dram_tensor(
        "dcp", [128, C * NCH * 128], io_dt, kind="ExternalInput"
    ).ap()
    if npow:
        pw_d = nc.dram_tensor(
            "pows", [128, npow * NCH * 128], io_dt, kind="ExternalInput"
        ).ap()
    hr_d = nc.dram_tensor("hr", [C, 128, NCH * RCOLS], f32, kind="ExternalOutput").ap()

    # zero-pad in front of the chunk axis so the multi-tap shifted reads in
    # phase B (shift up to npow chunks) and the carry shift in phase C fall
    # into zeros instead of needing edge cases
    pad = BLOC * max(npow, 1)

    # pools shared across reps: identical per-rep instruction stream, but
    # the scheduler can overlap rep r+1's prefetch DMAs with rep r's tail
    with tile.TileContext(nc) as tc:
      with (
            tc.tile_pool(name="wts", bufs=64) as wpool,
            tc.tile_pool(name="hl", bufs=C * NCH) as hlpool,
            tc.tile_pool(name="xt", bufs=2 * NCH) as xtpool,
            tc.tile_pool(name="hp", bufs=2 * NCH) as hppool,
            tc.tile_pool(name="z", bufs=2 * NCH) as zpool,
            tc.tile_pool(name="out", bufs=2 * NCH) as outpool,
            tc.tile_pool(name="ps", bufs=8, space="PSUM") as pspool,
        ):
       # --- HAM warmup: the PE clock-gate sits at K=4/8 (1.2 GHz) until
       # ~3.4us of sustained matmul activity. The first real matmul waits
       # on the x/weight DMAs anyway, so burn that idle window on dummy
       # matmuls to flip the gate before real work starts. Single-shot
       # executions then run warm; repeated executions are unaffected.
       if os.environ.get("RNN_WARMUP", "1") == "1":
            wu_w = wpool.tile([128, 128], io_dt, tag="wuw", bufs=1)
            wu_x = xtpool.tile([128, 128], io_dt, tag="wux", bufs=1)
            nc.gpsimd.memset(vin(wu_w[:]), 0.0)
            nc.gpsimd.memset(vin(wu_x[:]), 0.0)
            wu_ps = pspool.tile([128, RCOLS], f32, tag="ps")
            for _ in range(24):
                nc.tensor.matmul(
                    wu_ps[:, 0:128], wu_w[:], wu_x[:], start=True, stop=True
                )
       for _rep in range(reps):
            # --- resident weights: ONE wide DMA per tensor; views slice out
            # the stationary 128x128 blocks ---
            def load_flat(src, name, ncols):
                t = wpool.tile([128, ncols], io_dt, tag=name, bufs=1)
                nc.sync.dma_start(out=t[:], in_=src[:, :])
                return t

            def band_views(t):
                # t[:, a*U + b*128 : ...] = block (row-band a, col-block b)
                return [
                    [t[:, a * U + 128 * b : a * U + 128 * (b + 1)] for b in range(NCH)]
                    for a in range(NCH)
                ]

            def load_x(c):
                t = xtpool.tile([128, NCH * RCOLS], io_dt, tag="x", bufs=3)
                nc.sync.dma_start(out=t[:], in_=xr_d[c])
                return [t[:, RCOLS * d : RCOLS * (d + 1)] for d in range(NCH)]

            wp_t = band_views(load_flat(wp_d, "wp", NCH * U))
            dbt = load_flat(db_d, "db", NCH * 128)
            d_t = [dbt[:, 128 * u : 128 * (u + 1)] for u in range(NCH)]
            xpre = {0: load_x(0), 1: load_x(1)}
            pv_t = band_views(load_flat(pv_d, "pv", NCH * U))
            pw_t = []
            dcp_t = []

            # --- phase A: intra-chunk local scan in z-coordinates ---
            hl = [[None] * NCH for _ in range(C)]
            for c in range(C):
                xts = xpre.pop(c) if c in xpre else load_x(c)
                if c == 2 and npow:
                    pwt = load_flat(pw_d, "pw", npow * NCH * 128)
                    pw_t = [
                        [
                            pwt[:, (k * NCH + u) * 128 : (k * NCH + u + 1) * 128]
                            for u in range(NCH)
                        ]
                        for k in range(npow)
                    ]
                if c == 3:
                    # phase-C correction powers D^(c+1), diagonal blocks
                    dcpt = load_flat(dcp_d, "dcp", C * NCH * 128)
                    dcp_t = [
                        [
                            dcpt[:, (cc * NCH + u) * 128 : (cc * NCH + u + 1) * 128]
                            for u in range(NCH)
                        ]
                        for cc in range(C)
                    ]
                for u in range(NCH):
                    ops = [(wp_t[d][u][:], xts[d][:]) for d in range(NCH)]
                    if c > 0:
                        ops.append((d_t[u][:], hl[c - 1][u][:]))
                    ps = pspool.tile([128, RCOLS], f32, tag="ps")
                    for i, (lhsT, rhs) in enumerate(ops):
                        nc.tensor.matmul(
                            ps[:], lhsT, rhs,
                            start=(i == 0), stop=(i == len(ops) - 1),
                        )
                    # critical-path copies stay on DVE only: the scalar queue
                    # stalls on DMA-ring backpressure from its dma_starts
                    ht = hlpool.tile([128, RCOLS], io_dt, tag=f"hl{c}_{u}", bufs=1)
                    nc.vector.tensor_copy(out=ht[:], in_=ps[:])
                    hl[c][u] = ht

            if PHASES == "a":
                # debug build: dump zloc as the output, skip B/C
                for c in range(C):
                    ot = outpool.tile([128, NCH * RCOLS], f32, tag="o", bufs=2)
                    for u in range(NCH):
                        nc.vector.tensor_copy(
                            out=ot[:, RCOLS * u : RCOLS * (u + 1)], in_=vin(hl[c][u][:])
                        )
                    nc.sync.dma_start(out=hr_d[c], in_=ot[:])
            else:
                # --- phase B: chunk-end carries, single round of npow taps:
                # carry_j = end_j + sum_k end_{j-k} @ D^(C*k)  (k=1..npow;
                # dropped tail has ||D^(C*(npow+1))|| <= POW_TOL) ---
                hpa, hpb = [], []
                for v in range(NCH):
                    ta = hppool.tile([128, pad + RCOLS], io_dt, tag=f"hpa{v}", bufs=1)
                    tb = hppool.tile([128, BLOC + RCOLS], io_dt, tag=f"hpb{v}", bufs=1)
                    nc.gpsimd.memset(vin(ta[:, 0:pad]), 0.0)
                    nc.gpsimd.memset(vin(tb[:, 0:BLOC]), 0.0)
                    nc.vector.tensor_copy(
                        out=ta[:, pad : pad + RCOLS], in_=vin(hl[C - 1][v][:])
                    )
                    hpa.append(ta)
                    hpb.append(tb)
                for u in range(NCH):
                    ps = pspool.tile([128, RCOLS], f32, tag="ps")
                    for k in range(npow):
                        sh = BLOC * (k + 1)
                        nc.tensor.matmul(
                            ps[:], pw_t[k][u][:],
                            hpa[u][:, pad - sh : pad - sh + RCOLS],
                            start=(k == 0), stop=(k == npow - 1),
                        )
                    nc.vector.tensor_add(
                        out=hpb[u][:, BLOC : BLOC + RCOLS], in0=ps[:],
                        in1=vin(hpa[u][:, pad : pad + RCOLS]),
                    )
                src = hpb

                if PHASES == "ab":
                    for c in range(C):
                        ot = outpool.tile([128, NCH * RCOLS], f32, tag="o", bufs=2)
                        for u in range(NCH):
                            nc.vector.tensor_copy(
                                out=ot[:, RCOLS * u : RCOLS * (u + 1)],
                                in_=vin(hl[c][u][:]),
                            )
                        nc.sync.dma_start(out=hr_d[c], in_=ot[:])
                else:
                    # --- phase C: apply carries, transform, emit h ---
                    # carry views stay static; correction for step c is
                    # carry @ D^(c+1) via host powers (no serial G chain)
                    carry = [src[v][:, 0:RCOLS] for v in range(NCH)]
                    def drain(dc, pss):
                        # PSUM -> one wide SBUF tile -> single DRAM DMA
                        ot = outpool.tile([128, NCH * RCOLS], f32, tag="o", bufs=2)
                        for u in range(NCH):
                            dst = ot[:, RCOLS * u : RCOLS * (u + 1)]
                            if u < 2:
                                nc.vector.tensor_copy(out=dst, in_=pss[u][:])
                            else:
                                nc.scalar.copy(out=dst, in_=pss[u][:])
                        nc.sync.dma_start(out=hr_d[dc], in_=ot[:])

                    pend = None
                    for c in range(C):
                        zts = []
                        for u in range(NCH):
                            psg = pspool.tile([128, RCOLS], f32, tag="ps")
                            nc.tensor.matmul(
                                psg[:], dcp_t[c][u][:], carry[u],
                                start=True, stop=True,
                            )
                            # z_c = zloc_c + carry @ D^(c+1)
                            zt = zpool.tile([128, RCOLS], io_dt, tag=f"z{u}", bufs=2)
                            nc.vector.tensor_add(
                                out=zt[:], in0=psg[:], in1=vin(hl[c][u][:])
                            )
                            zts.append(zt)
                        # drain the PREVIOUS step's transform after this
                        # step's z-adds so DVE frees its PSUM banks in time
                        # without delaying the adds that gate the PE
                        if pend is not None:
                            drain(*pend)
                        pss = []
                        for u in range(NCH):
                            ps = pspool.tile([128, RCOLS], f32, tag="ps")
                            for v in range(NCH):
                                nc.tensor.matmul(
                                    ps[:], pv_t[v][u][:], zts[v][:],
                                    start=(v == 0), stop=(v == NCH - 1),
                                )
                            pss.append(ps)
                        pend = (c, pss)
                    drain(*pend)

    _split_sync_waits(nc)
    return nc


_CACHE = {}


def _get_nc(npow, reps=1):
    key = (npow, MM_DTYPE, PHASES, reps)
    if key not in _CACHE:
        _CACHE[key] = _build_nc(npow, reps)
    return _CACHE[key]


def _tf32_round(a):
    b = np.ascontiguousarray(a, np.float32).view(np.uint32)
    r = ((b >> np.uint32(13)) & np.uint32(1)) + np.uint32(0x0FFF)
    b = (b + r) & np.uint32(0xFFFFE000)
    return b.view(np.float32)


def _cast_host(a):
    if MM_DTYPE == "bf16":
        import ml_dtypes

        return np.ascontiguousarray(a.astype(ml_dtypes.bfloat16))
    if MM_DTYPE == "f16":
        return np.ascontiguousarray(np.asarray(a, np.float32).astype(np.float16))
    if MM_DTYPE == "f32r":
        return np.ascontiguousarray(_tf32_round(a))
    return np.ascontiguousarray(a.astype(np.float32))


def _block_diagonalize(r):
    """Split spec(R) into 4 conjugate-closed clusters of 128 dims (by
    eigenvalue angle) and orthonormalize each invariant subspace.
    Returns (P, Pinv, Dbd) fp64 with Dbd = P^-1 R P exactly
    block-diagonal (dense 128x128 diagonal blocks)."""
    r = np.asarray(r, np.float64)
    lam, V = np.linalg.eig(r)
    modes = []
    for i in range(len(lam)):
        if abs(lam[i].imag) < 1e-12:
            modes.append((1, 0.0 if lam[i].real > 0 else np.pi, [V[:, i].real]))
        elif lam[i].imag > 0:
            modes.append(
                (2, abs(np.angle(lam[i])), [V[:, i].real, V[:, i].imag])
            )
    modes.sort(key=lambda m: m[1])
    remaining = list(modes)
    clusters = []
    for g in range(NCH):
        cl, dsum = [], 0
        i = 0
        while dsum < 128 and i <= len(remaining):
            if i == len(remaining):
                break
            m = remaining[i]
            if dsum + m[0] <= 128:
                cl.append(m)
                dsum += m[0]
                remaining.pop(i)
            else:
                j = next(
                    (jj for jj in range(i, len(remaining)) if remaining[jj][0] == 1),
                    None,
                )
                if j is None:
                    break
                cl.append(remaining[j])
                dsum += 1
                remaining.pop(j)
        assert dsum == 128, (g, dsum)
        clusters.append(cl)
    assert not remaining
    Qs = []
    for cl in clusters:
        cols = np.stack([col for m in cl for col in m[2]], axis=1)
        q, _ = np.linalg.qr(cols)
        Qs.append(q)
    P = np.concatenate(Qs, axis=1)
    Pinv = np.linalg.inv(P)
    Dfull = Pinv @ r @ P
    Dbd = np.zeros_like(Dfull)
    for a in range(NCH):
        s = slice(128 * a, 128 * (a + 1))
        Dbd[s, s] = Dfull[s, s]
    return P, Pinv, Dbd


def prepare_inputs(x, kernel, recurrent_kernel):
    """Host-side decomposition + shard + permute. Returns (in_maps, npow)."""
    x = np.asarray(x)
    kernel = np.asarray(kernel, np.float64)
    P, Pinv, Dbd = _block_diagonalize(recurrent_kernel)
    def flat_bands(mat):
        # [D, U] -> [128, (D/128)*U]: row-bands side by side, partition-major
        return np.concatenate(
            [mat[128 * a : 128 * (a + 1), :] for a in range(mat.shape[0] // 128)],
            axis=1,
        )

    def flat_diag(mat):
        # diagonal 128-blocks side by side -> [128, NCH*128]
        return np.concatenate(
            [
                mat[128 * u : 128 * (u + 1), 128 * u : 128 * (u + 1)]
                for u in range(NCH)
            ],
            axis=1,
        )

    # multi-tap carry powers D^(C*k), k=1..npow (block-diagonal); stop when
    # the next tap's norm is below tolerance (contractive spectrum).
    pows = []
    dC = np.linalg.matrix_power(Dbd, C)
    m = dC
    while np.linalg.norm(m, 2) > POW_TOL and len(pows) < L - 1:
        pows.append(m)
        m = m @ dC
    npow = len(pows)
    pw = (
        _cast_host(np.concatenate([flat_diag(p) for p in pows], axis=1))
        if npow
        else None
    )
    wp = _cast_host(flat_bands(kernel @ P))
    pv = _cast_host(flat_bands(Pinv))
    db = _cast_host(flat_diag(Dbd))
    # phase-C correction powers D^(c+1), c = 0..C-1 (block-diagonal)
    dcp = _cast_host(
        np.concatenate(
            [flat_diag(np.linalg.matrix_power(Dbd, c + 1)) for c in range(C)], axis=1
        )
    )
    in_maps = []
    for k in range(NCORES):
        xc = x[BLOC * k : BLOC * (k + 1)]  # [BLOC, T, D]
        # xr[c, p, d*RCOLS + j*BLOC + b] = xc[b, j*C + c, d*128 + p]
        xr = _cast_host(
            xc.reshape(BLOC, L, C, NCH, 128)
            .transpose(2, 4, 3, 1, 0)
            .reshape(C, 128, NCH * RCOLS)
        )
        im = {"xr": xr, "wp": wp, "pv": pv, "db": db, "dcp": dcp}
        if npow:
            im["pows"] = pw
        in_maps.append(im)
    return in_maps, npow


def assemble_output(results):
    out = np.empty((B, T, U), np.float32)
    for k in range(NCORES):
        hr = results[k]["hr"]  # [C, 128, NCH*RCOLS]
        # out[b, j*C + c, u*128 + p] = hr[c, p, u*RCOLS + j*BLOC + b]
        out[BLOC * k : BLOC * (k + 1)] = (
            hr.reshape(C, 128, NCH, L, BLOC)
            .transpose(4, 3, 0, 2, 1)
            .reshape(BLOC, T, U)
        )
    return out


_RUNNERS = {}


def _get_runner(nc):
    """Build (once) a sharded jitted executable for `nc` on 8 cores.
    Mirrors bass2jax.run_bass_via_pjrt's multi-core path, but cached so
    repeated kernel() calls don't re-trace/re-compile."""
    if nc in _RUNNERS:
        return _RUNNERS[nc]
    import jax
    from jax.sharding import Mesh, PartitionSpec
    from jax.experimental.shard_map import shard_map
    from concourse import bass2jax

    bass2jax.install_neuronx_cc_hook()
    partition_name = nc.partition_id_tensor.name if nc.partition_id_tensor else None
    in_names, out_names, out_avals = [], [], []
    for alloc in nc.m.functions[0].allocations:
        if not isinstance(alloc, mybir.MemoryLocationSet):
            continue
        name = alloc.memorylocations[0].name
        if alloc.kind == "ExternalInput":
            if name != partition_name:
                in_names.append(name)
        elif alloc.kind == "ExternalOutput":
            out_names.append(name)
            out_avals.append(
                jax.core.ShapedArray(
                    tuple(alloc.tensor_shape), mybir.dt.np(alloc.dtype)
                )
            )
    n_params = len(in_names)
    in_names_all = list(in_names) + out_names
    if partition_name is not None:
        in_names_all.append(partition_name)

    def _body(*args):
        operands = list(args)
        if partition_name is not None:
            operands.append(bass2jax.partition_id_tensor())
        return tuple(
            bass2jax._bass_exec_p.bind(
                *operands,
                out_avals=tuple(out_avals),
                in_names=tuple(in_names_all),
                out_names=tuple(out_names),
                lowering_input_output_aliases=(),
                sim_require_finite=True,
                sim_require_nnan=True,
                nc=nc,
            )
        )

    devices = jax.devices()[:NCORES]
    mesh = Mesh(np.asarray(devices), ("core",))
    nouts = len(out_names)
    sharded = jax.jit(
        shard_map(
            _body,
            mesh=mesh,
            in_specs=(PartitionSpec("core"),) * (n_params + nouts),
            out_specs=(PartitionSpec("core"),) * nouts,
            check_rep=False,
        ),
        keep_unused=True,
    )

    def run(in_maps):
        concat_in = [
            np.concatenate([np.asarray(in_maps[c][nm]) for c in range(NCORES)], axis=0)
            for nm in in_names
        ]
        concat_zero = [
            np.zeros((NCORES * a.shape[0], *a.shape[1:]), a.dtype) for a in out_avals
        ]
        outs = sharded(*concat_in, *concat_zero)
        return [
            {
                nm: np.asarray(outs[i]).reshape(NCORES, *out_avals[i].shape)[c]
                for i, nm in enumerate(out_names)
            }
            for c in range(NCORES)
        ]

    run.sharded = sharded
    run.in_names = list(in_names)
    run.out_shapes = [(tuple(a.shape), a.dtype) for a in out_avals]
    _RUNNERS[nc] = run
    return run


def kernel(x, kernel, recurrent_kernel):
    in_maps, npow = prepare_inputs(x, kernel, recurrent_kernel)
    nc = _get_nc(npow)
    results = _get_runner(nc)(in_maps)
    return assemble_output(results)


# revision 36
# speedup vs baseline: 1.0449x; 1.0408x over previous
"""MinimalRNNCell on 8 Trainium2 NeuronCores.

h_t = x_t @ W + h_{t-1} @ R, h_0 = 0, for x: [B=32, T=1024, D=512],
W: [D, U=512], R: [U, U]. Returns all h_t -> [B, T, U] float32.

Strategy (data-parallel over batch; chunked linear scan in a
block-diagonalizing eigenbasis):
  - Host: eigendecompose R (fp64), group the spectrum into 4 conjugate-
    closed clusters of 128 dims (sorted by eigenvalue angle), take an
    orthonormal basis Q_g of each invariant subspace. P = [Q_1..Q_4]
    gives D = P^-1 R P exactly block-diagonal with dense 128x128 blocks
    and cond(P) ~ 2e2 (vs ~5e3 for the full modal form, which fails
    tf32 accuracy). The scan runs in z = h @ P coordinates:
        z_t = x_t @ (W P) + z_{t-1} @ D,   h_t = z_t @ P^-1.
    Recurrence matmuls touch only the 4 diagonal blocks -> 4x cheaper
    than dense R, at the cost of one extra dense transform (z @ P^-1)
    fused into the output drain. Net PE work: 324 vs 400 [128x128x512]
    matmuls for the dense-R chunked scan.
  - Shard batch over 8 cores (4 rows each). All matmul work runs in the
    transposed layout z^T[U, r]; host pre-permutes x into xr[c, d, r]
    with r = (chunk j, batch b), t = j*C + c, so every DMA is
    contiguous.
  - Phase A: C=8 sequential steps; step c advances all L=128 chunks at
    once: zloc_c = x_c @ (WP) + zloc_{c-1} @ D as one PSUM group per
    128-row block (4 WP matmuls + 1 diag-block matmul).
  - Phase B: chunk-boundary carries in ONE round of npow taps,
    carry_j = end_j + sum_k end_{j-k} @ D^(C*k), with host powers
    (block-diagonal -> 1 matmul per tap per 128-block, no serial
    doubling rounds). Contractive spectrum -> ~3 taps survive POW_TOL.
  - Phase C: C steps; correction carry @ D^(c+1) via host powers (all
    steps independent - no serial G chain), z_c = zloc_c + corr fused
    on DVE, then the dense h_c = z_c @ P^-1 transform streams out.

Scheduling notes (from NTFF device traces): every input tensor is ONE
partition-major flat DMA (a stalled dma_start on a compute queue
head-of-line blocks the copies phase B needs -> 9us PE bubble + HAM
re-throttle); PSUM->SBUF copies that gate matmuls live on the vector
queue only; the transform drain is software-pipelined one step behind
the correction matmuls. fp16 throughout (same 10-bit mantissa as tf32
here, half the DMA bytes, FWL halves LDWEIGHTS, 2x DVE).

Matmul dtype is selectable via RNN_MM_DTYPE: "f32" (exact, 4 cyc/row),
"f32r" (TF32), "f16" (default), "bf16".
"""

import os

import numpy as np

import concourse.bass as bass
import concourse.mybir as mybir
import concourse.tile as tile
from concourse import bass_utils

B, T, D, U = 32, 1024, 512, 512
NCORES = 8
BLOC = B // NCORES  # 4 batch rows per core
C = 8  # intra-chunk steps (phase A/C length)
L = T // C  # 128 chunks
RCOLS = BLOC * L  # 512 moving columns
NCH = U // 128  # 4 partition chunks of the 512-dim
POW_TOL = 3e-4  # drop carry taps with ||D^(C*k)||_2 below this
MAX_SYNC_WAITS = 1

MM_DTYPE = os.environ.get("RNN_MM_DTYPE", "f16")
# debug: which phases to build ("a", "ab", "abc" = full kernel)
PHASES = os.environ.get("RNN_PHASES", "abc")


def _split_sync_waits(nc, max_waits=MAX_SYNC_WAITS):
    """Walrus rejects instructions carrying more than a couple of sync
    waits (CTRL structs in this toolchain cap out below what Tile's
    final drain needs). Hoist excess waits onto single-wait NoOps
    placed immediately before the offending instruction."""
    for fn in nc.m.functions:
        for bb in fn.blocks:
            insts = bb.instructions
            out, changed = [], False
            for inst in insts:
                si = inst.sync_info
                waits = list(si.on_wait) if si is not None else []
                if len(waits) > max_waits:
                    for k, w in enumerate(waits[:-max_waits]):
                        out.append(
                            mybir.InstNoOp(
                                name=f"I-wsplit-{inst.name}-{k}",
                                engine=inst.engine,
                                ins=[],
                                outs=[],
                                sync_info=mybir.SyncInfo(on_wait=[w], on_update=[]),
                            )
                        )
                    inst.sync_info = mybir.SyncInfo(
                        on_wait=waits[-max_waits:], on_update=list(si.on_update)
                    )
                    changed = True
                out.append(inst)
            if changed:
                insts[:] = out


def _build_nc(npow, reps=1):
    f32 = mybir.dt.float32
    if MM_DTYPE == "bf16":
        io_dt = mybir.dt.bfloat16
    elif MM_DTYPE == "f16":
        io_dt = mybir.dt.float16
    elif MM_DTYPE == "f32r":
        io_dt = mybir.dt.float32r
    else:
        io_dt = f32

    def vin(ap):
        # DVE/ACT read of an f32r tile: same bits as f32
        return ap.bitcast(f32) if MM_DTYPE == "f32r" else ap

    nc = bass.Bass("TRN2", target_bir_lowering=False, debug=False)
    # all inputs partition-major and pre-flattened so each tensor (and each
    # x step) is ONE wide DMA: few triggers, no queue head-of-line blocking
    xr_d = nc.dram_tensor("xr", [C, 128, NCH * RCOLS], io_dt, kind="ExternalInput").ap()
    wp_d = nc.dram_tensor("wp", [128, NCH * U], io_dt, kind="ExternalInput").ap()
    pv_d = nc.dram_tensor("pv", [128, NCH * U], io_dt, kind="ExternalInput").ap()
    db_d = nc.dram_tensor("db", [128, NCH * 128], io_dt, kind="ExternalInput").ap()
    dcp_d = nc.dram_tensor(
        "dcp", [128, C * NCH * 128], io_dt, kind="ExternalInput"
    ).ap()
    if npow:
        pw_d = nc.dram_tensor(
            "pows", [128, npow * NCH * 128], io_dt, kind="ExternalInput"
        ).ap()
    hr_d = nc.dram_tensor("hr", [C, 128, NCH * RCOLS], f32, kind="ExternalOutput").ap()

    # zero-pad in front of the chunk axis so the multi-tap shifted reads in
    # phase B (shift up to npow chunks) and the carry shift in phase C fall
    # into zeros instead of needing edge cases
    pad = BLOC * max(npow, 1)

    # pools shared across reps: identical per-rep instruction stream, but
    # the scheduler can overlap rep r+1's prefetch DMAs with rep r's tail
    with tile.TileContext(nc) as tc:
      with (
            tc.tile_pool(name="wts", bufs=64) as wpool,
            tc.tile_pool(name="hl", bufs=C * NCH) as hlpool,
            tc.tile_pool(name="xt", bufs=2 * NCH) as xtpool,
            tc.tile_pool(name="hp", bufs=2 * NCH) as hppool,
            tc.tile_pool(name="z", bufs=2 * NCH) as zpool,
            tc.tile_pool(name="out", bufs=2 * NCH) as outpool,
            tc.tile_pool(name="ps", bufs=8, space="PSUM") as pspool,
        ):
       # --- HAM warmup: the PE clock-gate sits at K=4/8 (1.2 GHz) until
       # ~3.4us of sustained matmul activity. The first real matmul waits
       # on the x/weight DMAs anyway, so burn that idle window on dummy
       # matmuls to flip the gate before real work starts. Single-shot
       # executions then run warm; repeated executions are unaffected.
       if os.environ.get("RNN_WARMUP", "1") == "1":
            wu_w = wpool.tile([128, 128], io_dt, tag="wuw", bufs=1)
            wu_x = xtpool.tile([128, 128], io_dt, tag="wux", bufs=1)
            nc.gpsimd.memset(vin(wu_w[:]), 0.0)
            nc.gpsimd.memset(vin(wu_x[:]), 0.0)
            wu_ps = pspool.tile([128, RCOLS], f32, tag="ps")
            for _ in range(48):
                nc.tensor.matmul(
                    wu_ps[:, 0:128], wu_w[:], wu_x[:], start=True, stop=True
                )
       for _rep in range(reps):
            # --- resident weights: ONE wide DMA per tensor; views slice out
            # the stationary 128x128 blocks ---
            def load_flat(src, name, ncols):
                t = wpool.tile([128, ncols], io_dt, tag=name, bufs=1)
                nc.sync.dma_start(out=t[:], in_=src[:, :])
                return t

            def band_views(t):
                # t[:, a*U + b*128 : ...] = block (row-band a, col-block b)
                return [
                    [t[:, a * U + 128 * b : a * U + 128 * (b + 1)] for b in range(NCH)]
                    for a in range(NCH)
                ]

            def load_x(c):
                t = xtpool.tile([128, NCH * RCOLS], io_dt, tag="x", bufs=3)
                nc.sync.dma_start(out=t[:], in_=xr_d[c])
                return [t[:, RCOLS * d : RCOLS * (d + 1)] for d in range(NCH)]

            wp_t = band_views(load_flat(wp_d, "wp", NCH * U))
            dbt = load_flat(db_d, "db", NCH * 128)
            d_t = [dbt[:, 128 * u : 128 * (u + 1)] for u in range(NCH)]
            xpre = {0: load_x(0), 1: load_x(1)}
            pv_t = band_views(load_flat(pv_d, "pv", NCH * U))
            pw_t = []
            dcp_t = []

            # --- phase A: intra-chunk local scan in z-coordinates ---
            hl = [[None] * NCH for _ in range(C)]
            for c in range(C):
                xts = xpre.pop(c) if c in xpre else load_x(c)
                if c == 2 and npow:
                    pwt = load_flat(pw_d, "pw", npow * NCH * 128)
                    pw_t = [
                        [
                            pwt[:, (k * NCH + u) * 128 : (k * NCH + u + 1) * 128]
                            for u in range(NCH)
                        ]
                        for k in range(npow)
                    ]
                if c == 3:
                    # phase-C correction powers D^(c+1), diagonal blocks
                    dcpt = load_flat(dcp_d, "dcp", C * NCH * 128)
                    dcp_t = [
                        [
                            dcpt[:, (cc * NCH + u) * 128 : (cc * NCH + u + 1) * 128]
                            for u in range(NCH)
                        ]
                        for cc in range(C)
                    ]
                for u in range(NCH):
                    ops = [(wp_t[d][u][:], xts[d][:]) for d in range(NCH)]
                    if c > 0:
                        ops.append((d_t[u][:], hl[c - 1][u][:]))
                    ps = pspool.tile([128, RCOLS], f32, tag="ps")
                    for i, (lhsT, rhs) in enumerate(ops):
                        nc.tensor.matmul(
                            ps[:], lhsT, rhs,
                            start=(i == 0), stop=(i == len(ops) - 1),
                        )
                    # critical-path copies stay on DVE only: the scalar queue
                    # stalls on DMA-ring backpressure from its dma_starts
                    ht = hlpool.tile([128, RCOLS], io_dt, tag=f"hl{c}_{u}", bufs=1)
                    nc.vector.tensor_copy(out=ht[:], in_=ps[:])
                    hl[c][u] = ht

            if PHASES == "a":
                # debug build: dump zloc as the output, skip B/C
                for c in range(C):
                    ot = outpool.tile([128, NCH * RCOLS], f32, tag="o", bufs=2)
                    for u in range(NCH):
                        nc.vector.tensor_copy(
                            out=ot[:, RCOLS * u : RCOLS * (u + 1)], in_=vin(hl[c][u][:])
                        )
                    nc.sync.dma_start(out=hr_d[c], in_=ot[:])
            else:
                # --- phase B: chunk-end carries, single round of npow taps:
                # carry_j = end_j + sum_k end_{j-k} @ D^(C*k)  (k=1..npow;
                # dropped tail has ||D^(C*(npow+1))|| <= POW_TOL) ---
                hpa, hpb = [], []
                for v in range(NCH):
                    ta = hppool.tile([128, pad + RCOLS], io_dt, tag=f"hpa{v}", bufs=1)
                    tb = hppool.tile([128, BLOC + RCOLS], io_dt, tag=f"hpb{v}", bufs=1)
                    nc.gpsimd.memset(vin(ta[:, 0:pad]), 0.0)
                    nc.gpsimd.memset(vin(tb[:, 0:BLOC]), 0.0)
                    nc.vector.tensor_copy(
                        out=ta[:, pad : pad + RCOLS], in_=vin(hl[C - 1][v][:])
                    )
                    hpa.append(ta)
                    hpb.append(tb)
                for u in range(NCH):
                    ps = pspool.tile([128, RCOLS], f32, tag="ps")
                    for k in range(npow):
                        sh = BLOC * (k + 1)
                        nc.tensor.matmul(
                            ps[:], pw_t[k][u][:],
                            hpa[u][:, pad - sh : pad - sh + RCOLS],
                            start=(k == 0), stop=(k == npow - 1),
                        )
                    nc.vector.tensor_add(
                        out=hpb[u][:, BLOC : BLOC + RCOLS], in0=ps[:],
                        in1=vin(hpa[u][:, pad : pad + RCOLS]),
                    )
                src = hpb

                if PHASES == "ab":
                    for c in range(C):
                        ot = outpool.tile([128, NCH * RCOLS], f32, tag="o", bufs=2)
                        for u in range(NCH):
                            nc.vector.tensor_copy(
                                out=ot[:, RCOLS * u : RCOLS * (u + 1)],
                                in_=vin(hl[c][u][:]),
                            )
                        nc.sync.dma_start(out=hr_d[c], in_=ot[:])
                else:
                    # --- phase C: apply carries, transform, emit h ---
                    # carry views stay static; correction for step c is
                    # carry @ D^(c+1) via host powers (no serial G chain)
                    carry = [src[v][:, 0:RCOLS] for v in range(NCH)]
                    def drain(dc, pss):
                        # PSUM -> one wide SBUF tile -> single DRAM DMA
                        ot = outpool.tile([128, NCH * RCOLS], f32, tag="o", bufs=2)
                        for u in range(NCH):
                            dst = ot[:, RCOLS * u : RCOLS * (u + 1)]
                            if u < 2:
                                nc.vector.tensor_copy(out=dst, in_=pss[u][:])
                            else:
                                nc.scalar.copy(out=dst, in_=pss[u][:])
                        nc.sync.dma_start(out=hr_d[dc], in_=ot[:])

                    # correction matmuls run one step AHEAD of the transform:
                    # the PE queue executes in order, so the independent
                    # step-c+1 corrections must sit in front of the z-add-
                    # gated step-c transform group, not behind it
                    def corr_mms(cc):
                        ps4 = []
                        for u in range(NCH):
                            psg = pspool.tile([128, RCOLS], f32, tag="ps")
                            nc.tensor.matmul(
                                psg[:], dcp_t[cc][u][:], carry[u],
                                start=True, stop=True,
                            )
                            ps4.append(psg)
                        return ps4

                    psgs = corr_mms(0)
                    pend = None
                    for c in range(C):
                        zts = []
                        for u in range(NCH):
                            # z_c = zloc_c + carry @ D^(c+1)
                            zt = zpool.tile([128, RCOLS], io_dt, tag=f"z{u}", bufs=2)
                            nc.vector.tensor_add(
                                out=zt[:], in0=psgs[u][:], in1=vin(hl[c][u][:])
                            )
                            zts.append(zt)
                        psgs = corr_mms(c + 1) if c + 1 < C else None
                        # drain the PREVIOUS step's transform after this
                        # step's z-adds so DVE frees its PSUM banks in time
                        # without delaying the adds that gate the PE
                        if pend is not None:
                            drain(*pend)
                        pss = []
                        for u in range(NCH):
                            ps = pspool.tile([128, RCOLS], f32, tag="ps")
                            for v in range(NCH):
                                nc.tensor.matmul(
                                    ps[:], pv_t[v][u][:], zts[v][:],
                                    start=(v == 0), stop=(v == NCH - 1),
                                )
                            pss.append(ps)
                        pend = (c, pss)
                    drain(*pend)

    _split_sync_waits(nc)
    return nc


_CACHE = {}


def _get_nc(npow, reps=1):
    key = (npow, MM_DTYPE, PHASES, reps)
    if key not in _CACHE:
        _CACHE[key] = _build_nc(npow, reps)
    return _CACHE[key]


def _tf32_round(a):
    b = np.ascontiguousarray(a, np.float32).view(np.uint32)
    r = ((b >> np.uint32(13)) & np.uint32(1)) + np.uint32(0x0FFF)
    b = (b + r) & np.uint32(0xFFFFE000)
    return b.view(np.float32)


def _cast_host(a):
    if MM_DTYPE == "bf16":
        import ml_dtypes

        return np.ascontiguousarray(a.astype(ml_dtypes.bfloat16))
    if MM_DTYPE == "f16":
        return np.ascontiguousarray(np.asarray(a, np.float32).astype(np.float16))
    if MM_DTYPE == "f32r":
        return np.ascontiguousarray(_tf32_round(a))
    return np.ascontiguousarray(a.astype(np.float32))


def _block_diagonalize(r):
    """Split spec(R) into 4 conjugate-closed clusters of 128 dims (by
    eigenvalue angle) and orthonormalize each invariant subspace.
    Returns (P, Pinv, Dbd) fp64 with Dbd = P^-1 R P exactly
    block-diagonal (dense 128x128 diagonal blocks)."""
    r = np.asarray(r, np.float64)
    lam, V = np.linalg.eig(r)
    modes = []
    for i in range(len(lam)):
        if abs(lam[i].imag) < 1e-12:
            modes.append((1, 0.0 if lam[i].real > 0 else np.pi, [V[:, i].real]))
        elif lam[i].imag > 0:
            modes.append(
                (2, abs(np.angle(lam[i])), [V[:, i].real, V[:, i].imag])
            )
    modes.sort(key=lambda m: m[1])
    remaining = list(modes)
    clusters = []
    for g in range(NCH):
        cl, dsum = [], 0
        i = 0
        while dsum < 128 and i <= len(remaining):
            if i == len(remaining):
                break
            m = remaining[i]
            if dsum + m[0] <= 128:
                cl.append(m)
                dsum += m[0]
                remaining.pop(i)
            else:
                j = next(
                    (jj for jj in range(i, len(remaining)) if remaining[jj][0] == 1),
                    None,
                )
                if j is None:
                    break
                cl.append(remaining[j])
                dsum += 1
                remaining.pop(j)
        assert dsum == 128, (g, dsum)
        clusters.append(cl)
    assert not remaining
    Qs = []
    for cl in clusters:
        cols = np.stack([col for m in cl for col in m[2]], axis=1)
        q, _ = np.linalg.qr(cols)
        Qs.append(q)
    P = np.concatenate(Qs, axis=1)
    Pinv = np.linalg.inv(P)
    Dfull = Pinv @ r @ P
    Dbd = np.zeros_like(Dfull)
    for a in range(NCH):
        s = slice(128 * a, 128 * (a + 1))
        Dbd[s, s] = Dfull[s, s]
    return P, Pinv, Dbd


def prepare_inputs(x, kernel, recurrent_kernel):
    """Host-side decomposition + shard + permute. Returns (in_maps, npow)."""
    x = np.asarray(x)
    kernel = np.asarray(kernel, np.float64)
    P, Pinv, Dbd = _block_diagonalize(recurrent_kernel)
    def flat_bands(mat):
        # [D, U] -> [128, (D/128)*U]: row-bands side by side, partition-major
        return np.concatenate(
            [mat[128 * a : 128 * (a + 1), :] for a in range(mat.shape[0] // 128)],
            axis=1,
        )

    def flat_diag(mat):
        # diagonal 128-blocks side by side -> [128, NCH*128]
        return np.concatenate(
            [
                mat[128 * u : 128 * (u + 1), 128 * u : 128 * (u + 1)]
                for u in range(NCH)
            ],
            axis=1,
        )

    # multi-tap carry powers D^(C*k), k=1..npow (block-diagonal); stop when
    # the next tap's norm is below tolerance (contractive spectrum).
    pows = []
    dC = np.linalg.matrix_power(Dbd, C)
    m = dC
    while np.linalg.norm(m, 2) > POW_TOL and len(pows) < L - 1:
        pows.append(m)
        m = m @ dC
    npow = len(pows)
    pw = (
        _cast_host(np.concatenate([flat_diag(p) for p in pows], axis=1))
        if npow
        else None
    )
    wp = _cast_host(flat_bands(kernel @ P))
    pv = _cast_host(flat_bands(Pinv))
    db = _cast_host(flat_diag(Dbd))
    # phase-C correction powers D^(c+1), c = 0..C-1 (block-diagonal)
    dcp = _cast_host(
        np.concatenate(
            [flat_diag(np.linalg.matrix_power(Dbd, c + 1)) for c in range(C)], axis=1
        )
    )
    in_maps = []
    for k in range(NCORES):
        xc = x[BLOC * k : BLOC * (k + 1)]  # [BLOC, T, D]
        # xr[c, p, d*RCOLS + j*BLOC + b] = xc[b, j*C + c, d*128 + p]
        xr = _cast_host(
            xc.reshape(BLOC, L, C, NCH, 128)
            .transpose(2, 4, 3, 1, 0)
            .reshape(C, 128, NCH * RCOLS)
        )
        im = {"xr": xr, "wp": wp, "pv": pv, "db": db, "dcp": dcp}
        if npow:
            im["pows"] = pw
        in_maps.append(im)
    return in_maps, npow


def assemble_output(results):
    out = np.empty((B, T, U), np.float32)
    for k in range(NCORES):
        hr = results[k]["hr"]  # [C, 128, NCH*RCOLS]
        # out[b, j*C + c, u*128 + p] = hr[c, p, u*RCOLS + j*BLOC + b]
        out[BLOC * k : BLOC * (k + 1)] = (
            hr.reshape(C, 128, NCH, L, BLOC)
            .transpose(4, 3, 0, 2, 1)
            .reshape(BLOC, T, U)
        )
    return out


_RUNNERS = {}


def _get_runner(nc):
    """Build (once) a sharded jitted executable for `nc` on 8 cores.
    Mirrors bass2jax.run_bass_via_pjrt's multi-core path, but cached so
    repeated kernel() calls don't re-trace/re-compile."""
    if nc in _RUNNERS:
        return _RUNNERS[nc]
    import jax
    from jax.sharding import Mesh, PartitionSpec
    from jax.experimental.shard_map import shard_map
    from concourse import bass2jax

    bass2jax.install_neuronx_cc_hook()
    partition_name = nc.partition_id_tensor.name if nc.partition_id_tensor else None
    in_names, out_names, out_avals = [], [], []
    for alloc in nc.m.functions[0].allocations:
        if not isinstance(alloc, mybir.MemoryLocationSet):
            continue
        name = alloc.memorylocations[0].name
        if alloc.kind == "ExternalInput":
            if name != partition_name:
                in_names.append(name)
        elif alloc.kind == "ExternalOutput":
            out_names.append(name)
            out_avals.append(
                jax.core.ShapedArray(
                    tuple(alloc.tensor_shape), mybir.dt.np(alloc.dtype)
                )
            )
    n_params = len(in_names)
    in_names_all = list(in_names) + out_names
    if partition_name is not None:
        in_names_all.append(partition_name)

    def _body(*args):
        operands = list(args)
        if partition_name is not None:
            operands.append(bass2jax.partition_id_tensor())
        return tuple(
            bass2jax._bass_exec_p.bind(
                *operands,
                out_avals=tuple(out_avals),
                in_names=tuple(in_names_all),
                out_names=tuple(out_names),
                lowering_input_output_aliases=(),
                sim_require_finite=True,
                sim_require_nnan=True,
                nc=nc,
            )
        )

    devices = jax.devices()[:NCORES]
    mesh = Mesh(np.asarray(devices), ("core",))
    nouts = len(out_names)
    sharded = jax.jit(
        shard_map(
            _body,
            mesh=mesh,
            in_specs=(PartitionSpec("core"),) * (n_params + nouts),
            out_specs=(PartitionSpec("core"),) * nouts,
            check_rep=False,
        ),
        keep_unused=True,
    )

    def run(in_maps):
        concat_in = [
            np.concatenate([np.asarray(in_maps[c][nm]) for c in range(NCORES)], axis=0)
            for nm in in_names
        ]
        concat_zero = [
            np.zeros((NCORES * a.shape[0], *a.shape[1:]), a.dtype) for a in out_avals
        ]
        outs = sharded(*concat_in, *concat_zero)
        return [
            {
                nm: np.asarray(outs[i]).reshape(NCORES, *out_avals[i].shape)[c]
                for i, nm in enumerate(out_names)
            }
            for c in range(NCORES)
        ]

    run.sharded = sharded
    run.in_names = list(in_names)
    run.out_shapes = [(tuple(a.shape), a.dtype) for a in out_avals]
    _RUNNERS[nc] = run
    return run


def kernel(x, kernel, recurrent_kernel):
    in_maps, npow = prepare_inputs(x, kernel, recurrent_kernel)
    nc = _get_nc(npow)
    results = _get_runner(nc)(in_maps)
    return assemble_output(results)
